# revision 65
# baseline (speedup 1.0000x reference)
"""Trainium2 Bass kernel for nn_Conv2D_26164940767465.

Per-(channel, filter) VALID 2D cross-correlation with NO channel reduction:
  out[b, ho, c, f, wo] = sum_{i,j} int(x[b, ho+i, wo+j, c]) * int(k[i,j,c,f])

Shapes: x (4,224,224,16) f32 integer-valued [0,256); k (5,5,16,32) f32
integer-valued [-8,8). Output (4,220,16,32,220) f32.

Exactness: x <= 255 and |k| <= 8 are exactly representable in bf16; products
(<= 2040) and 25-tap sums (|.| <= 51000 < 2^24) are exact in the fp32 PSUM
accumulator. The store downcasts to fp16 (max 65504 > 51000, 11-bit
mantissa): per-element relative error <= 2^-11 ~ 4.9e-4, far inside the
2e-2 gate; the host upcasts to fp32. fp16 halves the store traffic
(49.6 -> 24.8 MB/core), which was the v1 pacer.

Strategy (8 cores): shard (batch 4) x (output-row halves 2). Per core the
input lives in SBUF once, as a j-shifted channel-major buffer XSJ: partition
32g + c_l*5 + j holds row-major image rows of channel c = 4*g + c_l shifted
left by j. Only the 20 used partitions per group exist in DRAM (80 total,
4.0MB vs 6.6MB padded), streamed onto the chip via the GpSimd SWDGE queue
from a piece-major layout (see PIECE_* below). For every pair of output
rows, the four channel groups g run CONCURRENTLY as 4 row-tiled matmuls
(K=20 each, tile_position=(32g,0)) into 4 PSUM banks, accumulating the 5
kernel rows i via rhs free-offset (r+i)*WO — no im2col materialization at
all. PSUM tiles are evacuated by vector/scalar engines and written out in
10-row chunks whose per-partition DRAM runs are fully contiguous 8800B
descriptors (out layout [C, F, rows, wo]), alternating the two HWDGE store
queues by group parity.

With fp32 stores this structure was store-DMA-bound (~417 GB/s sustained on
49.6MB/core); with fp16 stores the pacer is the tensor engine's
LDWEIGHTS+MATMUL issue pipeline (~110ns per pair; walrus emits one
LDWEIGHTS per matmul unconditionally - its redundant-load optimization
crashes and is hard-disabled - and each 128-col weight load costs
~1.1ns/col on the weight XBUS). Restructurings that cut the matmul count by
folding i-taps into K (K=40, 2 row-shifted input copies, 3 accumulation
steps) were measured SLOWER (213us): K>32 halves the row-tile concurrency,
and per-strip streams cap at N/1.2GHz. The 4-lane/K=20/5-step shape with
double-buffered PSUM (4 tags x bufs=2) is the optimum of that landscape.
The DMA design choices remain: (a) every SDMA engine equally loaded (HWDGE
descriptors are partition-owned: partitions 8e..8e+7 belong to engine e),
(b) store queues saturated early (split first-chunk stores), (c) DMA
completion semaphores prompt (issue-order ring throttling turns any slow
completion into an engine stall upstream).
"""

import os
import sys

if "/opt/trn_rl_repo" not in sys.path:
    sys.path.insert(0, "/opt/trn_rl_repo")

import numpy as np
import ml_dtypes

BF16 = np.dtype(ml_dtypes.bfloat16)

# Problem constants (hardcoded per harness contract).
B, H, W, C = 4, 224, 224, 16
KH, KW, F = 5, 5, 32
HO, WO = H - KH + 1, W - KW + 1          # 220, 220
NCORES = 8
HALF = HO // 2                            # 110 output rows per core
HIN = HALF + KH - 1                       # 114 input rows per core
CG = 4                                    # channels per group
NG = C // CG                              # 4 groups
KJ = CG * KW                              # 20 contraction rows per group
MP = CG * F                               # 128 output partitions
ROWS_PER_MM = 2
NFREE = ROWS_PER_MM * WO                  # 440
PSB = 512                                 # fp32 cols per PSUM bank
# Chunks staged per output DMA: 20-row chunks (8800B fp16 descriptors)
# mid-kernel, graduating to 10-row chunks for the last 30 rows. A chunk's
# bytes can only start storing once its last PSUM copy lands, so big chunks
# near the end lump ~4.5MB behind the final matmuls; smaller end chunks
# keep the SDMA drain continuous and cut the post-compute tail.
CHUNKS = [(0, 20), (20, 20), (40, 20), (60, 20),
          (80, 10), (90, 10), (100, 10)]

# SBUF partition base for each group's 20-row input window. Ldweights
# requires the partition base at the quadrant boundary, so windows are
# [32g, 32g+20). Loading only these 80 partitions (instead of all 128 with
# 48 zero rows) cuts input HBM traffic 6.58->4.0 MB.
PB = [0, 32, 64, 96]
# The input rows are stored PIECE-MAJOR in DRAM ([piece, part, 8 rows]): a
# single dma_start then emits one ~3.5KB descriptor per (partition, piece) —
# small enough for the fast SWDGE path — independent of how many pieces the
# DMA covers. (Per-partition runs >4KB run at ~9GB/s on SWDGE; separate
# 8-row dma_starts choke on the ~2us/DMA throttled SWDGE issue rate.)
#
# Everything rides SWDGE: each SWDGE DMA is a single <=64-descriptor packet
# pinned to ONE engine, so a concurrent wide HWDGE load's completion
# semaphore (which needs all 16 engines to drain) straggles by ~10us —
# putting any latency-critical load on HWDGE while the stream runs was
# repeatedly measured 15-35us slower.
PIECE_ROWS = 8
NPIECES = 15                              # rows 0..120 (6 zero pad rows)
# Stream DMAs cover piece ranges: single-piece blocks first (small packets
# = short single-engine hog = fast unlock of chunk 0), then 2-piece blocks.
PIECE_SPLITS = [0, 1, 2, 4, 6, 8, 10, 12, 14, NPIECES]

_PROGRAM = None


def _build_program():
    import concourse.bacc as bacc
    import concourse.mybir as mybir
    import concourse.tile as tile

    nc = bacc.Bacc("TRN2", target_bir_lowering=False, debug=False,
                   num_devices=NCORES)

    # wt is padded to the full 116-partition span (zero rows in the
    # inter-group gaps) so it loads as ONE wide DMA — issued on Scalar
    # before any SWDGE packet exists, so its completion is clean.
    xsj_d = nc.dram_tensor("xsj", [NPIECES, 4 * KJ, PIECE_ROWS * WO],
                           mybir.dt.bfloat16, kind="ExternalInput")
    wt_d = nc.dram_tensor("wt", [PB[-1] + KJ, KH * MP], mybir.dt.bfloat16,
                          kind="ExternalInput")
    # [C, F, rows, wo] layout: each output partition (c_l, f) owns a fully
    # contiguous DRAM run per chunk. Host transposes back on assembly.
    out_d = nc.dram_tensor("out", [C, F, HALF, WO], mybir.dt.float16,
                           kind="ExternalOutput")

    xsj_ap = xsj_d.ap()
    wt_ap = wt_d.ap()
    out_ap = out_d.ap()

    with tile.TileContext(nc) as tc:
        with (
            tc.tile_pool(name="wpool", bufs=1) as wpool,
            tc.tile_pool(name="xpool", bufs=1) as xpool,
            tc.tile_pool(name="spool", bufs=4) as spool,
            tc.tile_pool(name="psum", bufs=2, space="PSUM") as pspool,
        ):
            # Inputs on the GpSimd SWDGE queue. SWDGE sprays descriptors
            # round-robin over all 16 SDMA engines instead of the fixed
            # partition->engine ownership of HWDGE, so the input bytes (which
            # live only on partitions 0-115) don't turn engines 0-9 into
            # the kernel-makespan long pole — every HWDGE placement tried
            # (scalar, sync) lost 15-35us to that imbalance plus ring-FIFO
            # interference with the stores. GpSimd issues nothing else, so
            # its ring-capacity throttle waits block nobody.
            wt_t = wpool.tile([128, KH * MP], mybir.dt.bfloat16)
            nc.scalar.dma_start(wt_t[0:PB[-1] + KJ, :], wt_ap[:, :])
            xsj_t = xpool.tile([128, NPIECES * PIECE_ROWS * WO],
                               mybir.dt.bfloat16)
            # Block-outer issue order: all 4 groups of piece-block b land
            # before block b+1, matching the chunks' consumption order.
            # The final piece holds only 2 real rows (112-113); the 6 zero
            # pad rows are never read by matmuls, so don't transfer them.
            for a, b in zip(PIECE_SPLITS, PIECE_SPLITS[1:]):
                for g in range(NG):
                    sview = xsj_t[PB[g]:PB[g] + KJ, :] \
                        .rearrange("p (pc f) -> p pc f", pc=NPIECES)
                    dview = xsj_ap[:, g * KJ:(g + 1) * KJ, :] \
                        .rearrange("pc p f -> p pc f")
                    if b == NPIECES:
                        nc.gpsimd.dma_start(
                            sview[:, a:b, 0:(HIN - a * PIECE_ROWS) * WO],
                            dview[:, a:b, 0:(HIN - a * PIECE_ROWS) * WO])
                    else:
                        nc.gpsimd.dma_start(sview[:, a:b, :],
                                            dview[:, a:b, :])

            # HAM warm-up. Measured: row-tiled matmuls run at exactly
            # N/1.2GHz (the cold clock-gate rate) and never trip the PE
            # activity monitor into the 2.4GHz state on their own. After
            # ~4us of full-array (K=116) matmuls, tiled matmul cadence
            # drops 367 -> 186ns (= N/2.4 + NX overhead) for the ~13us
            # until the HAM's idle window re-throttles. These run on the
            # already-loaded weight tile during the dead input-fill window
            # (outputs are never read) and buy ~3-4us. Sustaining the warm
            # state was measured a net loss: tiny full-array "blips" every
            # 2 pairs stall the 4-strip pipeline ~4us each (full-array
            # matmuls are a pipeline barrier) and a 2% duty cycle doesn't
            # satisfy the activity monitor anyway (+15us total); dummy
            # re-warm bursts cost ~as much dead time as the warm window
            # saves.
            for wu in range(12):
                wps = pspool.tile([MP, PSB], mybir.dt.float32,
                                  tag="ps0", name="ps0")
                nc.tensor.matmul(
                    wps[:, 0:NFREE], wt_t[0:116, 0:MP],
                    wt_t[0:116, 200:200 + NFREE],
                    start=True, stop=True, tile_position=(0, 0),
                )

            pair_ctr = 0
            for r0c, crows in CHUNKS:
                # Fixed 20-row allocation regardless of chunk size so the
                # pool's rotating buffers keep one shape; short chunks use
                # a prefix slice.
                stages = [
                    spool.tile([MP, 20 * WO], mybir.dt.float16,
                               tag=f"stage{g}", name=f"stage{g}")
                    for g in range(NG)
                ]
                # N=440 row-pair matmuls: the per-strip rhs stream issues at
                # a hard 1.2 GHz (cadence exactly N/1.2 = 367ns; LDWEIGHTS
                # hides under the previous matmul), and N=512 slab variants
                # measured STRICTLY worse (+24us) - bigger N does not
                # amortize anything, it just streams at the same rate with
                # an extra penalty.
                for t in range(crows // ROWS_PER_MM):
                    r = r0c + t * ROWS_PER_MM
                    # Periodic HAM re-warm bursts: a burst measurably
                    # re-warms the clock gate mid-stream (one at pair 10
                    # produced a 48us warm window, 40-88us all at 186ns
                    # cadence). Warm windows vary (13-48us observed), so
                    # burst every 10 pairs (~12.3us warm-time) to stay
                    # under the shortest observed window. A burst costs
                    # ~2.5-3us when already warm; a warm pair saves ~0.6us
                    # over cold. Even MM count keeps the ps0 pool rotation
                    # parity intact.
                    if pair_ctr in (10, 33):
                        for wu in range(12):
                            wps = pspool.tile([MP, PSB], mybir.dt.float32,
                                              tag="ps0", name="ps0")
                            nc.tensor.matmul(
                                wps[:, 0:NFREE], wt_t[0:116, 0:MP],
                                wt_t[0:116, 200:200 + NFREE],
                                start=True, stop=True,
                                tile_position=(0, 0),
                            )
                    pss = [
                        pspool.tile([MP, PSB], mybir.dt.float32,
                                    tag=f"ps{g}", name=f"ps{g}")
                        for g in range(NG)
                    ]
                    pair_ctr += 1
                    for i in range(KH):
                        off = (r + i) * WO
                        for g in range(NG):
                            nc.tensor.matmul(
                                pss[g][:, 0:NFREE],
                                wt_t[PB[g]:PB[g] + KJ, i * MP:(i + 1) * MP],
                                xsj_t[PB[g]:PB[g] + KJ, off:off + NFREE],
                                start=(i == 0), stop=(i == KH - 1),
                                tile_position=(32 * g, 0),
                            )
                    for g in range(NG):
                        dst = stages[g][:, t * NFREE:(t + 1) * NFREE]
                        if (t * NG + g) % 2 == 0:
                            nc.vector.tensor_copy(dst, pss[g][:, 0:NFREE])
                        else:
                            nc.scalar.copy(dst, pss[g][:, 0:NFREE])
                # First chunk: split stores so draining starts while the
                # chunk's last pairs are still being copied (pipeline fill).
                # Other chunks: one 20-row store per group — 8800B fp16
                # descriptors, and the end-of-kernel drain is
                # issue-serialized by the ring throttle (~2.6us completion
                # latency per DMA), so the final stores must be big to keep
                # the queues fed; the last chunk is 10 rows so the
                # post-compute drain is half-length.
                if r0c == 0:
                    splits = ((0, 4), (4, 12), (12, 20))
                elif r0c == 100:
                    # Final chunk: split the stores so rows 100-105 drain
                    # while the last two pairs still compute - only 0.9MB
                    # trails the final matmul instead of the whole 2.25MB.
                    # Keep the sync/scalar alternation: piling the split
                    # stores onto one queue serializes their issues past
                    # the last matmul (measured +4us), and flooding scalar
                    # with per-pair stores ring-stalls its remaining PSUM
                    # copies and the tensor quartets (measured +10us).
                    splits = ((0, 6), (6, 10))
                else:
                    splits = ((0, crows),)
                for g in range(NG):
                    eng = nc.sync if g % 2 == 0 else nc.scalar
                    sview = stages[g][:].rearrange("p (r w) -> p r w", w=WO)
                    for a, b in splits:
                        dram_slab = out_ap[
                            g * CG:(g + 1) * CG, :, r0c + a:r0c + b, :,
                        ].rearrange("c f r w -> (c f) r w")
                        eng.dma_start(dram_slab, sview[:, a:b, :])

    nc.compile()
    return nc


def _get_program():
    global _PROGRAM
    if _PROGRAM is None:
        _PROGRAM = _build_program()
    return _PROGRAM


def _host_pack(x, k):
    """Build per-core XSJ tensors and the shared per-tap weights (bf16)."""
    x_bf = np.ascontiguousarray(x.astype(BF16))
    k_bf = k.astype(BF16)

    np_ = PB[-1] + KJ
    xsj_all = []
    for m in range(NCORES):
        b, half = m // 2, m % 2
        r0 = half * HALF
        # Deinterleave once: [C, 114, 224] channel-major rows.
        xc = np.ascontiguousarray(x_bf[b, r0:r0 + HIN].transpose(2, 0, 1))
        # Piece-major stream: [piece, partition, 8 rows x WO]; rows beyond
        # HIN stay zero.
        xp = np.zeros((NPIECES * PIECE_ROWS, 4 * KJ, WO), dtype=BF16)
        for c in range(C):
            g, cl = c // CG, c % CG
            for j in range(KW):
                xp[:HIN, KJ * g + cl * KW + j] = xc[c, :, j:j + WO]
        xsj_all.append(np.ascontiguousarray(
            xp.reshape(NPIECES, PIECE_ROWS, 4 * KJ, WO)
            .transpose(0, 2, 1, 3)).reshape(NPIECES, 4 * KJ,
                                            PIECE_ROWS * WO))

    wt = np.zeros((np_, KH, MP), dtype=BF16)
    for c in range(C):
        g, cl = c // CG, c % CG
        base = PB[g] + cl * KW
        for j in range(KW):
            for i in range(KH):
                wt[base + j, i, cl * F:(cl + 1) * F] = k_bf[i, j, c, :]
    wt = np.ascontiguousarray(wt.reshape(np_, KH * MP))
    return xsj_all, wt


LAST_EXEC_TIME_NS = None


def kernel(**inputs):
    from concourse.bass_utils import run_bass_kernel_spmd

    global LAST_EXEC_TIME_NS
    x = np.asarray(inputs["inputs"])
    k = np.asarray(inputs["kernel"])
    assert x.shape == (B, H, W, C) and k.shape == (KH, KW, C, F)

    nc = _get_program()
    xsj_all, wt = _host_pack(x, k)
    in_maps = [{"xsj": xsj_all[m], "wt": wt} for m in range(NCORES)]

    trace = os.environ.get("CONV_TRACE", "") == "1"
    kwargs = {}
    if trace:
        kwargs["trace"] = True
        tdir = os.environ.get("CONV_TRACE_DIR")
        if tdir:
            kwargs["tmpdir"] = tdir

    res = run_bass_kernel_spmd(nc, in_maps, list(range(NCORES)), **kwargs)
    LAST_EXEC_TIME_NS = res.exec_time_ns

    full = np.empty((B, HO, C, F, WO), dtype=np.float32)
    for m in range(NCORES):
        b, half = m // 2, m % 2
        # device layout [C, F, rows, WO] fp16 -> reference layout
        # [rows, C, F, WO] fp32
        full[b, half * HALF:(half + 1) * HALF] = \
            res.results[m]["out"].transpose(2, 0, 1, 3).astype(np.float32)
    return full

